# revision 2
# baseline (speedup 1.0000x reference)
"""BotRGCN + MoE (top-1 of 2 experts) Trainium2 Bass kernel, 8-core SPMD. v2

Design (changes vs v1 marked *):
  - Nodes sharded contiguously: core c owns nodes [c*6250, (c+1)*6250).
  - Activations processed in TRANSPOSED windows: (128 part = d-half, 2 d-chunks,
    <=512 nodes free); full transposed activations staged in per-core DRAM.
  - RGCN aggregation = aggregate-then-transform:
      s[seg] = sum_{e: seg_e=seg} x_full[src_e],  seg = rel*6250 + local_dst
      agg^T = sum_r W_rel[r]^T @ (s_r^T * cnt_inv)
    Segment sums via selector matmuls: per 128-edge chunk,
      psum[d_half, seg cols] += M_chunk(128 edges, d_half).T @ Sel(128 edges, gsize)
    with Sel = (iota == seg_local), built on DVE, 0/1 exact.
  - *Gather tables split in TWO chunks A/B at local row 3072 (window boundary):
    chunk A = rows {c*3072..} over cores, chunk B = the rest. Each chunk is
    AllGather'd separately: A fires mid-layer (after window 5's export), B at
    layer end -> the A collective overlaps the tail of the producing layer and
    next layer's A-gathers can start while B is still in flight. Both chunk
    tables are < 32768 rows so int16 gather indices work without a lo/hi split.
  - Gathers via gpsimd.dma_gather (int16 idx).
  - *cnt_inv input is [NSEGW, 1, WIN]; broadcast across partitions on device
    with a K=1 matmul into PSUM (was: host-replicated x128 = 6.8MB input).
  - *idx16 input is [16, IDXC]; replicated to [128, IDXC] on device via a
    single partition-wrapped DMA (was: host-replicated x8).
  - *MoE expert matmuls run in bf16 (gate logits stay fp32: top-1 selection
    is discontinuous, the x pipeline stays fp32 end-to-end for it).
  - *Output is written bf16 and upcast on host (2e-2 tolerance, saves D2H).
  - MoE via gated-h1: y^T = sum_e We2_e^T (g_e * lrelu(We1_e^T x + be1_e))
    + bias terms folded in as K=1 matmuls against the gate row.

Self-contained: hardcodes shapes; imports only installed packages.
"""

import os

import numpy as np

N = 50000
E = 400000
D = 256
R = 2
NE = 2
OUT = 256
NCORES = 8
NLOC = N // NCORES  # 6250
WIN = 512
NWIN = (NLOC + WIN - 1) // WIN  # 13
CHUNK = 128

# A/B table split at a window boundary (6 windows / 7 windows)
A_WIN = 6
A_LOC = A_WIN * WIN            # 3072 local rows in chunk A
B_LOC = NLOC - A_LOC           # 3178 in chunk B
NA = NCORES * A_LOC            # 24576 rows in table A
NB = NCORES * B_LOC            # 25424 rows in table B

N_GRID = int(os.environ.get("N_GRID", "96"))
SPLIT_MODE = os.environ.get("SPLIT_MODE", "lohi")  # "ab" (per-core A/B) | "lohi"
LO_SPLIT = int(os.environ.get("LO_SPLIT", "25000"))
MOE_BF16 = bool(int(os.environ.get("MOE_BF16", "1")))
OUT_BF16 = bool(int(os.environ.get("OUT_BF16", "1")))
OPT_GATHER = bool(int(os.environ.get("OPT_GATHER", "1")))
OPT_COLLECTIVE = bool(int(os.environ.get("OPT_COLLECTIVE", "1")))
COLL_AT_END = bool(int(os.environ.get("COLL_AT_END", "0")))
REPEAT = int(os.environ.get("REPEAT", "1"))

SELU_SCALE = 1.0507009873554805
SELU_ALPHA = 1.6732632423543772
NEG_SLOPE = 0.01


# ----------------------------------------------------------------------------
# host-side planning
# ----------------------------------------------------------------------------

def _wrap_idx(idx):
    """int16 index list (len multiple of 16) -> (16, len/16) wrapped."""
    n = len(idx)
    return np.ascontiguousarray(idx.reshape(n // 16, 16).T.astype(np.int16))


def build_plan(edge_index, edge_type):
    src = np.asarray(edge_index[0], dtype=np.int64)
    dst = np.asarray(edge_index[1], dtype=np.int64)
    rel = np.asarray(edge_type, dtype=np.int64)

    core = dst // NLOC
    seg = rel * NLOC + (dst % NLOC)
    NSEG = R * NLOC

    cnt = np.bincount((core * NSEG + seg).astype(np.int64),
                      minlength=NCORES * NSEG).reshape(NCORES, NSEG)

    # chunk-table index of each edge's source
    s_core = src // NLOC
    s_loc = src % NLOC
    if SPLIT_MODE == "ab":
        in_a = s_loc < A_LOC
        tbl_idx = np.where(in_a, s_core * A_LOC + s_loc,
                           s_core * B_LOC + (s_loc - A_LOC))
    else:
        in_a = src < LO_SPLIT
        tbl_idx = np.where(in_a, src, src - LO_SPLIT)

    # groups: (r, wbase, gbase, gsize, win_index)
    groups = []
    wi = 0
    for r in range(R):
        for w in range(NWIN):
            wb = w * WIN
            nw = min(WIN, NLOC - wb)
            g0 = 0
            while g0 < nw:
                gs = min(N_GRID, nw - g0)
                groups.append((r, wb, wb + g0, gs, wi))
                g0 += gs
            wi += 1
    n_windows_total = wi

    per_core = []
    for c in range(NCORES):
        m = core == c
        o = np.argsort(seg[m], kind="stable")
        per_core.append((tbl_idx[m][o], seg[m][o], in_a[m][o]))

    n_a = np.zeros(len(groups), np.int64)
    n_b = np.zeros(len(groups), np.int64)
    core_group_edges = []
    for c in range(NCORES):
        t_c, seg_c, a_c = per_core[c]
        lst = []
        for gi, (r, wb, gb, gs, _) in enumerate(groups):
            lo_b = np.searchsorted(seg_c, r * NLOC + gb)
            hi_b = np.searchsorted(seg_c, r * NLOC + gb + gs)
            tt = t_c[lo_b:hi_b]
            gg = seg_c[lo_b:hi_b] - (r * NLOC + gb)
            aa = a_c[lo_b:hi_b]
            ea = (tt[aa], gg[aa])
            eb = (tt[~aa], gg[~aa])
            lst.append((ea, eb))
            n_a[gi] = max(n_a[gi], -(-len(ea[0]) // CHUNK))
            n_b[gi] = max(n_b[gi], -(-len(eb[0]) // CHUNK))
        core_group_edges.append(lst)
    empty = (n_a + n_b) == 0
    n_a[empty] = 1

    win_groups = [[] for _ in range(n_windows_total)]
    for gi, g in enumerate(groups):
        win_groups[g[4]].append(gi)

    slot_group = []
    win_slot_range = []
    for w in range(n_windows_total):
        start = len(slot_group)
        na_w = 0
        for gi in win_groups[w]:
            slot_group.extend([gi] * int(n_a[gi]))
            na_w += int(n_a[gi])
        nb_w = 0
        for gi in win_groups[w]:
            slot_group.extend([gi] * int(n_b[gi]))
            nb_w += int(n_b[gi])
        win_slot_range.append((start, na_w, nb_w))
    n_slots = len(slot_group)

    idx_all = np.zeros((NCORES, n_slots, CHUNK), np.int16)
    seg_all = np.full((NCORES, CHUNK, n_slots), -1.0, np.float32)
    for c in range(NCORES):
        for w in range(n_windows_total):
            start, na_w, nb_w = win_slot_range[w]
            cursor = start
            for half in (0, 1):
                for gi in win_groups[w]:
                    nsl = int((n_a if half == 0 else n_b)[gi])
                    if nsl == 0:
                        continue
                    tt, gg = core_group_edges[c][gi][half]
                    ne = len(tt)
                    pad = nsl * CHUNK - ne
                    ttp = np.concatenate([tt, np.zeros(pad, np.int64)])
                    ggp = np.concatenate([gg, np.full(pad, -1, np.int64)])
                    for k in range(nsl):
                        sl = cursor + k
                        idx_all[c, sl] = ttp[k * CHUNK:(k + 1) * CHUNK].astype(np.int16)
                        seg_all[c, :, sl] = ggp[k * CHUNK:(k + 1) * CHUNK].astype(np.float32)
                    cursor += nsl

    win_idx_cols = []
    col = 0
    for w in range(n_windows_total):
        start, na_w, nb_w = win_slot_range[w]
        win_idx_cols.append((col, na_w * CHUNK, col + na_w * CHUNK // 16,
                             nb_w * CHUNK))
        col += (na_w + nb_w) * CHUNK // 16
    tot_cols = col

    idx16 = []
    for c in range(NCORES):
        buf = np.zeros((16, tot_cols), np.int16)
        for w in range(n_windows_total):
            start, na_w, nb_w = win_slot_range[w]
            c_a, ni_a, c_b, ni_b = win_idx_cols[w]
            if ni_a:
                buf[:, c_a:c_a + ni_a // 16] = _wrap_idx(
                    idx_all[c, start:start + na_w].reshape(-1))
            if ni_b:
                buf[:, c_b:c_b + ni_b // 16] = _wrap_idx(
                    idx_all[c, start + na_w:start + na_w + nb_w].reshape(-1))
        idx16.append(buf)

    cntinv = np.ones((NCORES, n_windows_total, 1, WIN), np.float32)
    for c in range(NCORES):
        for r in range(R):
            for w in range(NWIN):
                wb = w * WIN
                nw = min(WIN, NLOC - wb)
                cc = cnt[c, r * NLOC + wb: r * NLOC + wb + nw]
                cntinv[c, r * NWIN + w, 0, :nw] = 1.0 / np.maximum(cc, 1)

    plan = dict(
        groups=groups,
        slot_group=np.array(slot_group, np.int64),
        win_groups=win_groups,
        win_slot_range=win_slot_range,
        win_idx_cols=win_idx_cols,
        n_slots=n_slots,
        idx_cols=tot_cols,
        n_windows_total=n_windows_total,
        max_slots=max(ws[1] + ws[2] for ws in win_slot_range),
    )
    data = dict(idx16=idx16, seg_all=seg_all, cntinv=cntinv)
    return plan, data


# ----------------------------------------------------------------------------
# bass program
# ----------------------------------------------------------------------------

def build_nc(plan):
    import concourse.mybir as mybir
    import concourse.tile as tile
    from concourse import bacc
    from concourse.masks import make_identity

    dt = mybir.dt
    f32 = dt.float32
    bf16 = dt.bfloat16
    moe_dt = bf16 if MOE_BF16 else f32
    out_dt = bf16 if OUT_BF16 else f32
    Alu = mybir.AluOpType
    ACT = mybir.ActivationFunctionType

    NSEGW = plan["n_windows_total"]
    NSLOT = plan["n_slots"]
    IDXC = plan["idx_cols"]
    MAX_SLOTS = plan["max_slots"]
    groups = plan["groups"]
    slot_group = plan["slot_group"]

    nc = bacc.Bacc(None, num_devices=NCORES, num_swdge_queues=2)

    xcatT_in = nc.dram_tensor("xcatT", [128, 2, NLOC], f32, kind="ExternalInput")
    idx16_in = nc.dram_tensor("idx16", [16, IDXC], dt.int16, kind="ExternalInput")
    seg_in = nc.dram_tensor("segloc", [128, NSLOT], f32, kind="ExternalInput")
    cntinv_in = nc.dram_tensor("cntinv", [1, NSEGW, WIN], f32, kind="ExternalInput")
    w_in_in = nc.dram_tensor("w_in", [128, 2, D], f32, kind="ExternalInput")
    w_root_in = nc.dram_tensor("w_root", [128, 2, D], f32, kind="ExternalInput")
    w_rel_in = nc.dram_tensor("w_rel", [R, 128, 2, D], f32, kind="ExternalInput")
    b_in_in = nc.dram_tensor("b_in", [128, 2], f32, kind="ExternalInput")
    b_rgcn_in = nc.dram_tensor("b_rgcn", [128, 2], f32, kind="ExternalInput")
    wg_in = nc.dram_tensor("wgate", [128, 2, 1], f32, kind="ExternalInput")
    we1_in = nc.dram_tensor("we1", [NE, 128, 2, D], f32, kind="ExternalInput")
    be1_in = nc.dram_tensor("be1", [NE, 128, 2], f32, kind="ExternalInput")
    we2_in = nc.dram_tensor("we2", [NE, 128, 2, OUT], f32, kind="ExternalInput")
    be2_in = nc.dram_tensor("be2row", [1, NE, 2, 128], f32, kind="ExternalInput")
    out_t = nc.dram_tensor("out", [OUT, NLOC], out_dt, kind="ExternalOutput")

    with tile.TileContext(nc) as tc:
        with (
            tc.tile_pool(name="const", bufs=1) as cpool,
            tc.tile_pool(name="work", bufs=2) as wpool,
            tc.tile_pool(name="slabp", bufs=2) as slabpool,
            tc.tile_pool(name="selp", bufs=4) as selpool,
            tc.tile_pool(name="stage", bufs=3) as stpool,
            tc.tile_pool(name="psum_sel", bufs=2, space="PSUM") as ps_sel,
            tc.tile_pool(name="psum_xf", bufs=2, space="PSUM") as ps_xf,
            tc.tile_pool(name="psum_misc", bufs=2, space="PSUM") as ps_misc,
            tc.tile_pool(name="dram", bufs=1, space="DRAM") as dpool,
            tc.tile_pool(name="dramsh", bufs=1, space="DRAM") as shpool,
        ):
            # constants / weights
            ident = cpool.tile([128, 128], f32)
            make_identity(nc, ident[:])
            iota_i = cpool.tile([128, N_GRID], dt.int32)
            nc.gpsimd.iota(iota_i[:], pattern=[[1, N_GRID]], base=0,
                           channel_multiplier=0)
            iota_f = cpool.tile([128, N_GRID], f32)
            nc.vector.tensor_copy(iota_f[:], iota_i[:])
            ones_row = cpool.tile([1, 128], f32)
            nc.vector.memset(ones_row[:], 1.0)

            def load_const(t_in, shape, re=None, tag=None):
                t = cpool.tile(shape, f32, tag=tag)
                nc.sync.dma_start(t[:], t_in[:] if re is None else
                                  t_in[:].rearrange(re))
                return t

            w_in_sb = load_const(w_in_in, [128, 2, D], tag="w_in")
            w_root_sb = load_const(w_root_in, [128, 2, D], tag="w_root")
            w_rel_sb = load_const(w_rel_in, [128, R, 2, D], "r p k d -> p r k d",
                                  tag="w_rel")
            b_in_sb = load_const(b_in_in, [128, 2], tag="b_in")
            b_rg_sb = load_const(b_rgcn_in, [128, 2], tag="b_rg")
            wgd_sb = load_const(wg_in, [128, 2, 1], tag="wgd")
            be2_sb = cpool.tile([1, NE, 2, 128], f32)
            nc.sync.dma_start(be2_sb[:], be2_in[:])
            # MoE expert weights, cast to bf16 on the scalar engine
            we1_sb = cpool.tile([128, NE, 2, D], moe_dt, tag="we1")
            we2_sb = cpool.tile([128, NE, 2, OUT], moe_dt, tag="we2")
            for e in range(NE):
                for kc in range(2):
                    for t_in, t_sb in ((we1_in, we1_sb), (we2_in, we2_sb)):
                        tmp = stpool.tile([128, D], f32, tag="wcast")
                        nc.sync.dma_start(tmp[:], t_in[e, :, kc, :])
                        nc.scalar.activation(t_sb[:, e, kc, :], tmp[:],
                                             ACT.Copy)
            be1_sb = load_const(be1_in, [128, NE, 2], "e p k -> p e k", tag="be1")


            seg_sb = cpool.tile([128, NSLOT], f32)
            nc.sync.dma_start(seg_sb[:], seg_in[:])
            # idx: [16, IDXC] in DRAM -> [128, IDXC] sbuf (8x partition wrap)
            idx_sb = cpool.tile([128, IDXC], dt.int16)
            for _b in range(8):
                nc.sync.dma_start(idx_sb[16 * _b:16 * (_b + 1), :], idx16_in[:])

            cnt_bc = dpool.tile([NSEGW, 128, WIN], f32, tag="cntbc")
            for _wi in range(NSEGW):
                _cir = stpool.tile([1, WIN], f32, tag="cirow")
                nc.sync.dma_start(_cir[:], cntinv_in[0:1, _wi, :])
                _cip = ps_misc.tile([128, WIN], f32, space="PSUM", tag="misc")
                nc.tensor.matmul(_cip[:, :], ones_row[:], _cir[:, :],
                                 start=True, stop=True)
                _cis = stpool.tile([128, WIN], f32, tag="cisb")
                nc.scalar.activation(_cis[:], _cip[:], ACT.Copy)
                nc.sync.dma_start(cnt_bc[_wi], _cis[:])

            def make_stage(rep):
                xT1 = dpool.tile([128, 2, NLOC], f32, tag="xT1_%d" % rep)
                xT2 = dpool.tile([128, 2, NLOC], f32, tag="xT2_%d" % rep)
                xloc1 = dpool.tile([NLOC, D], f32, tag="xl1_%d" % rep)
                xloc2 = dpool.tile([NLOC, D], f32, tag="xl2_%d" % rep)
                if SPLIT_MODE == "ab":
                    xfA1 = shpool.tile([NA, D], f32, addr_space="Shared",
                                       tag="xfA1_%d" % rep)
                    xfB1 = shpool.tile([NB, D], f32, addr_space="Shared",
                                       tag="xfB1_%d" % rep)
                    xfA2 = shpool.tile([NA, D], f32, addr_space="Shared",
                                       tag="xfA2_%d" % rep)
                    xfB2 = shpool.tile([NB, D], f32, addr_space="Shared",
                                       tag="xfB2_%d" % rep)
                    return xT1, xT2, xloc1, xloc2, (xfA1, xfB1), (xfA2, xfB2)
                xfull1 = shpool.tile([N, D], f32, addr_space="Shared",
                                     tag="xfull1_%d" % rep)
                xfull2 = shpool.tile([N, D], f32, addr_space="Shared",
                                     tag="xfull2_%d" % rep)
                return (xT1, xT2, xloc1, xloc2,
                        (xfull1[0:LO_SPLIT, :], xfull1[LO_SPLIT:N, :], xfull1),
                        (xfull2[0:LO_SPLIT, :], xfull2[LO_SPLIT:N, :], xfull2))

            def win_sizes(w):
                wb = w * WIN
                return wb, min(WIN, NLOC - wb)

            def load_xwin(src_dram, wb, nw, tag):
                t = wpool.tile([128, 2, WIN], f32, tag=tag)
                nc.sync.dma_start(t[:, :, :nw], src_dram[:, :, wb:wb + nw])
                return t

            def export_window(xw, wb, nw, xloc):
                # transpose (128, 2, nw) -> node-major rows of xloc
                nb = 0
                while nb < nw:
                    bs = min(128, nw - nb)
                    stg = stpool.tile([128, D], f32, tag="stage")
                    for mc in range(2):
                        pst = ps_misc.tile([128, max(WIN, 128)], f32,
                                           space="PSUM", tag="misc")
                        nc.tensor.transpose(pst[:bs, :128], xw[:, mc, nb:nb + bs],
                                            ident[:])
                        nc.scalar.activation(stg[:bs, mc * 128:(mc + 1) * 128],
                                             pst[:bs, :128], ACT.Copy)
                    nc.sync.dma_start(xloc[wb + nb: wb + nb + bs, :], stg[:bs, :])
                    nb += bs

            def collect(xloc, xf_pair, half):
                """AllGather chunk `half` (0=A rows [0,A_LOC), 1=B) of xloc."""
                if not OPT_COLLECTIVE:
                    return
                import concourse.mybir as mybir
                if SPLIT_MODE != "ab":
                    if half == 0:
                        return  # single collective fires on half==1
                    nc.gpsimd.collective_compute(
                        "AllGather", mybir.AluOpType.bypass,
                        replica_groups=[list(range(NCORES))],
                        ins=[xloc[:].opt()], outs=[xf_pair[2][:].opt()])
                    return
                if half == 0:
                    nc.gpsimd.collective_compute(
                        "AllGather", mybir.AluOpType.bypass,
                        replica_groups=[list(range(NCORES))],
                        ins=[xloc[0:A_LOC, :].opt()], outs=[xf_pair[0][:].opt()])
                else:
                    nc.gpsimd.collective_compute(
                        "AllGather", mybir.AluOpType.bypass,
                        replica_groups=[list(range(NCORES))],
                        ins=[xloc[A_LOC:NLOC, :].opt()], outs=[xf_pair[1][:].opt()])

            # ---------------- layer 0: x1 = selu(x_cat @ W_in + b_in) -------
            def layer0(xT1, xloc1, xf1):
                for w in range(NWIN):
                    wb, nw = win_sizes(w)
                    xw = load_xwin(xcatT_in, wb, nw, "xw")
                    xo = wpool.tile([128, 2, WIN], f32, tag="xo")
                    for mc in range(2):
                        ps = ps_xf.tile([128, WIN], f32, space="PSUM", tag="xf")
                        for kc in range(2):
                            nc.tensor.matmul(
                                ps[:, :nw],
                                w_in_sb[:, kc, mc * 128:(mc + 1) * 128],
                                xw[:, kc, :nw],
                                start=(kc == 0), stop=(kc == 1),
                            )
                        pos = wpool.tile([128, WIN], f32, tag="selu_pos")
                        nc.vector.tensor_scalar(
                            out=pos[:, :nw], in0=ps[:, :nw],
                            scalar1=b_in_sb[:, mc:mc + 1], scalar2=0.0,
                            op0=Alu.add, op1=Alu.max)
                        neg = wpool.tile([128, WIN], f32, tag="selu_neg")
                        nc.vector.tensor_scalar(
                            out=neg[:, :nw], in0=ps[:, :nw],
                            scalar1=b_in_sb[:, mc:mc + 1], scalar2=0.0,
                            op0=Alu.add, op1=Alu.min)
                        e = wpool.tile([128, WIN], f32, tag="selu_e")
                        nc.scalar.activation(e[:, :nw], neg[:, :nw], ACT.Exp)
                        sa = SELU_SCALE * SELU_ALPHA
                        nc.vector.tensor_scalar(
                            out=e[:, :nw], in0=e[:, :nw], scalar1=sa, scalar2=sa,
                            op0=Alu.mult, op1=Alu.subtract)
                        nc.vector.tensor_scalar(
                            out=pos[:, :nw], in0=pos[:, :nw],
                            scalar1=SELU_SCALE, scalar2=None, op0=Alu.mult)
                        nc.vector.tensor_tensor(
                            out=xo[:, mc, :nw], in0=pos[:, :nw], in1=e[:, :nw],
                            op=Alu.add)
                    nc.sync.dma_start(xT1[:, :, wb:wb + nw], xo[:, :, :nw])
                    export_window(xo, wb, nw, xloc1)
                    if w == A_WIN - 1 and not COLL_AT_END:
                        collect(xloc1, xf1, 0)
                if COLL_AT_END:
                    collect(xloc1, xf1, 0)
                collect(xloc1, xf1, 1)

            # ---------------- rgcn layers ----------------
            def rgcn_layer(xf, xT_cur, xT_next, xloc_next, xf_next, li,
                           moe_fn=None):
                xfA, xfB = xf[0], xf[1]
                for w in range(NWIN):
                    wb, nw = win_sizes(w)
                    s_tiles = {}
                    for r in range(R):
                        wi = r * NWIN + w
                        start_slot, na_w, nb_w = plan["win_slot_range"][wi]
                        c_a, ni_a, c_b, ni_b = plan["win_idx_cols"][wi]
                        nslots_w = na_w + nb_w
                        slab = slabpool.tile([128, MAX_SLOTS, D], f32, tag="slab")
                        if ni_a and OPT_GATHER:
                            nc.gpsimd.dma_gather(
                                out_ap=slab[:, :na_w, :],
                                in_ap=xfA[:] if SPLIT_MODE == "ab" else xfA,
                                idxs_ap=idx_sb[:, c_a:c_a + ni_a // 16],
                                num_idxs=ni_a, num_idxs_reg=ni_a, elem_size=D,
                                single_packet=False, queue_num=r)
                        if ni_b and OPT_GATHER:
                            nc.gpsimd.dma_gather(
                                out_ap=slab[:, na_w:nslots_w, :],
                                in_ap=xfB[:] if SPLIT_MODE == "ab" else xfB,
                                idxs_ap=idx_sb[:, c_b:c_b + ni_b // 16],
                                num_idxs=ni_b, num_idxs_reg=ni_b, elem_size=D,
                                single_packet=False, queue_num=r)
                        ps0 = ps_sel.tile([128, WIN], f32, space="PSUM",
                                          tag="sel0")
                        ps1 = ps_sel.tile([128, WIN], f32, space="PSUM",
                                          tag="sel1")
                        for k in range(nslots_w):
                            sl = start_slot + k
                            gi = int(slot_group[sl])
                            gb_in_win = groups[gi][2] - wb
                            gs = groups[gi][3]
                            sel = selpool.tile([128, N_GRID], f32, tag="sel")
                            nc.vector.tensor_scalar(
                                out=sel[:, :gs], in0=iota_f[:, :gs],
                                scalar1=seg_sb[:, sl:sl + 1], scalar2=None,
                                op0=Alu.is_equal)
                            cols = slice(gb_in_win, gb_in_win + gs)
                            nc.tensor.matmul(
                                ps0[:, cols], slab[:, k, 0:128], sel[:, :gs],
                                start=(k == 0), stop=(k == nslots_w - 1))
                            nc.tensor.matmul(
                                ps1[:, cols], slab[:, k, 128:256], sel[:, :gs],
                                start=(k == 0), stop=(k == nslots_w - 1))
                        # broadcast cnt_inv row across partitions via K=1 matmul
                        ci_sb = wpool.tile([128, WIN], f32, tag="ci_%d" % r)
                        nc.sync.dma_start(ci_sb[:], cnt_bc[wi])
                        s0 = wpool.tile([128, WIN], f32, tag="s0_%d" % r)
                        s1 = wpool.tile([128, WIN], f32, tag="s1_%d" % r)
                        nc.vector.tensor_tensor(out=s0[:, :nw], in0=ps0[:, :nw],
                                                in1=ci_sb[:, :nw], op=Alu.mult)
                        nc.vector.tensor_tensor(out=s1[:, :nw], in0=ps1[:, :nw],
                                                in1=ci_sb[:, :nw], op=Alu.mult)
                        s_tiles[r] = (s0, s1)

                    xw = load_xwin(xT_cur, wb, nw, "xw")
                    xo = wpool.tile([128, 2, WIN], f32, tag="xo")
                    for mc in range(2):
                        ps = ps_xf.tile([128, WIN], f32, space="PSUM", tag="xf")
                        for kc in range(2):
                            nc.tensor.matmul(
                                ps[:, :nw],
                                w_root_sb[:, kc, mc * 128:(mc + 1) * 128],
                                xw[:, kc, :nw],
                                start=(kc == 0), stop=False)
                        for r in range(R):
                            for kc in range(2):
                                st = s_tiles[r][kc]
                                nc.tensor.matmul(
                                    ps[:, :nw],
                                    w_rel_sb[:, r, kc, mc * 128:(mc + 1) * 128],
                                    st[:, :nw],
                                    start=False, stop=(r == R - 1 and kc == 1))
                        nc.vector.tensor_scalar(
                            out=xo[:, mc, :nw], in0=ps[:, :nw],
                            scalar1=b_rg_sb[:, mc:mc + 1], scalar2=None,
                            op0=Alu.add)
                    if xT_next is not None:
                        nc.sync.dma_start(xT_next[:, :, wb:wb + nw],
                                          xo[:, :, :nw])
                    if xloc_next is not None:
                        export_window(xo, wb, nw, xloc_next)
                        if w == A_WIN - 1 and not COLL_AT_END:
                            collect(xloc_next, xf_next, 0)
                    if moe_fn is not None:
                        moe_fn(xo, wb, nw)
                if xloc_next is not None:
                    if COLL_AT_END:
                        collect(xloc_next, xf_next, 0)
                    collect(xloc_next, xf_next, 1)

            # ---------------- MoE (fused into layer 2 windows) ----------------
            def moe_window(xw, wb, nw):
                psl = ps_misc.tile([128, WIN], f32, space="PSUM", tag="misc")
                for kc in range(2):
                    nc.tensor.matmul(
                        psl[:1, :nw], wgd_sb[:, kc, :], xw[:, kc, :nw],
                        start=(kc == 0), stop=(kc == 1))
                g_row = wpool.tile([1, WIN], f32, tag="grow")
                nc.vector.tensor_scalar(out=g_row[:, :nw], in0=psl[:1, :nw],
                                        scalar1=0.0, scalar2=None, op0=Alu.is_ge)
                ginv_row = wpool.tile([1, WIN], f32, tag="ginvrow")
                nc.vector.tensor_scalar(out=ginv_row[:, :nw], in0=g_row[:, :nw],
                                        scalar1=-1.0, scalar2=1.0,
                                        op0=Alu.mult, op1=Alu.add)
                psb = ps_misc.tile([128, WIN], f32, space="PSUM", tag="misc")
                nc.tensor.matmul(psb[:, :nw], ones_row[:], g_row[:, :nw],
                                 start=True, stop=True)
                gb = wpool.tile([128, WIN], moe_dt, tag="gb_sb")
                nc.scalar.activation(gb[:, :nw], psb[:, :nw], ACT.Copy)
                ginv = wpool.tile([128, WIN], moe_dt, tag="ginv")
                nc.vector.tensor_scalar(out=ginv[:, :nw], in0=gb[:, :nw],
                                        scalar1=-1.0, scalar2=1.0,
                                        op0=Alu.mult, op1=Alu.add)
                # bf16 copy of the window activations for expert matmuls
                if MOE_BF16:
                    xb = wpool.tile([128, 2, WIN], moe_dt, tag="xb")
                    for mc in range(2):
                        nc.scalar.activation(xb[:, mc, :nw], xw[:, mc, :nw],
                                             ACT.Copy)
                else:
                    xb = xw

                h1g = {}
                for e in range(NE):
                    for mc in range(2):
                        psh = ps_xf.tile([128, WIN], f32, space="PSUM", tag="xf")
                        for kc in range(2):
                            nc.tensor.matmul(
                                psh[:, :nw],
                                we1_sb[:, e, kc, mc * 128:(mc + 1) * 128],
                                xb[:, kc, :nw],
                                start=(kc == 0), stop=(kc == 1))
                        h = wpool.tile([128, WIN], moe_dt,
                                       tag="h1_%d_%d" % (e, mc))
                        nc.scalar.activation(
                            h[:, :nw], psh[:, :nw], ACT.Lrelu,
                            bias=be1_sb[:, e, mc:mc + 1], alpha=NEG_SLOPE)
                        gt = gb if e == 0 else ginv
                        nc.vector.tensor_tensor(out=h[:, :nw], in0=h[:, :nw],
                                                in1=gt[:, :nw], op=Alu.mult)
                        h1g[(e, mc)] = h
                for mc in range(2):
                    psy = ps_xf.tile([128, WIN], f32, space="PSUM", tag="xf")
                    first = True
                    for e in range(NE):
                        for kc in range(2):
                            nc.tensor.matmul(
                                psy[:, :nw],
                                we2_sb[:, e, kc, mc * 128:(mc + 1) * 128],
                                h1g[(e, kc)][:, :nw],
                                start=first, stop=False)
                            first = False
                    nc.tensor.matmul(psy[:, :nw], be2_sb[0:1, 0, mc, :],
                                     g_row[:, :nw], start=False, stop=False)
                    nc.tensor.matmul(psy[:, :nw], be2_sb[0:1, 1, mc, :],
                                     ginv_row[:, :nw], start=False, stop=True)
                    yt = wpool.tile([128, WIN], out_dt, tag="yt")
                    nc.scalar.activation(yt[:, :nw], psy[:, :nw], ACT.Copy)
                    nc.sync.dma_start(out_t[mc * 128:(mc + 1) * 128, wb:wb + nw],
                                      yt[:, :nw])

            for _rep in range(REPEAT):
                xT1, xT2, xloc1, xloc2, xf1, xf2 = make_stage(_rep)
                layer0(xT1, xloc1, xf1)
                rgcn_layer(xf1, xT1, xT2, xloc2, xf2, 1)
                rgcn_layer(xf2, xT2, None, None, None, 2, moe_fn=moe_window)

    nc.compile()
    return nc


# ----------------------------------------------------------------------------
# entry point
# ----------------------------------------------------------------------------

def make_in_maps(inputs, data):
    des, tweet, num_prop, cat_prop = (np.asarray(inputs[k]) for k in
                                      ("des", "tweet", "num_prop", "cat_prop"))
    x_cat = np.concatenate([des, tweet, num_prop, cat_prop],
                           axis=1).astype(np.float32)

    def wmat(w):
        w = np.asarray(w, np.float32)
        return np.ascontiguousarray(w.reshape(2, 128, w.shape[1]).transpose(1, 0, 2))

    def bvec(b):
        return np.ascontiguousarray(np.asarray(b, np.float32).reshape(2, 128).T)

    W_rel, We1, be1, We2, be2 = (np.asarray(inputs[k]) for k in
                                 ("W_rel", "We1", "be1", "We2", "be2"))
    w_rel_h = np.stack([wmat(W_rel[r]) for r in range(R)])
    we1_h = np.stack([wmat(We1[e]) for e in range(NE)])
    be1_h = np.stack([bvec(be1[e]) for e in range(NE)])
    we2_h = np.stack([wmat(We2[e]) for e in range(NE)])
    be2row = np.asarray(be2, np.float32).reshape(1, NE, 2, 128)
    w_gate = np.asarray(inputs["w_gate"])

    in_maps = []
    for c in range(NCORES):
        xc = x_cat[c * NLOC:(c + 1) * NLOC]
        xcatT = np.ascontiguousarray(xc.T.reshape(2, 128, NLOC).transpose(1, 0, 2))
        in_maps.append({
            "xcatT": xcatT,
            "idx16": data["idx16"][c],
            "segloc": data["seg_all"][c],
            "cntinv": np.ascontiguousarray(
                data["cntinv"][c].reshape(1, -1, WIN)),
            "w_in": wmat(inputs["W_in"]), "w_root": wmat(inputs["W_root"]),
            "w_rel": w_rel_h,
            "b_in": bvec(inputs["b_in"]), "b_rgcn": bvec(inputs["b_rgcn"]),
            "wgate": wmat(w_gate[:, 0:1] - w_gate[:, 1:2]),
            "we1": we1_h, "be1": be1_h,
            "we2": we2_h, "be2row": be2row,
        })
    return in_maps


def kernel(des, tweet, num_prop, cat_prop, edge_index, edge_type,
           W_in, b_in, W_rel, W_root, b_rgcn, w_gate, We1, be1, We2, be2):
    from concourse.bass_utils import run_bass_kernel_spmd

    plan, data = build_plan(np.asarray(edge_index), np.asarray(edge_type))
    nc = build_nc(plan)
    in_maps = make_in_maps(dict(
        des=des, tweet=tweet, num_prop=num_prop, cat_prop=cat_prop,
        W_in=W_in, b_in=b_in, W_rel=W_rel, W_root=W_root, b_rgcn=b_rgcn,
        w_gate=w_gate, We1=We1, be1=be1, We2=We2, be2=be2), data)

    res = run_bass_kernel_spmd(nc, in_maps, core_ids=list(range(NCORES)))
    global last_nc, last_in_maps
    last_nc, last_in_maps = nc, in_maps
    y = np.concatenate(
        [np.asarray(res.results[c]["out"]).astype(np.float32).T
         for c in range(NCORES)], axis=0)
    return y.astype(np.float32)


last_nc = None
last_in_maps = None
